# revision 1
# baseline (speedup 1.0000x reference)
"""CCConvLayer (GNN message passing) on 8 Trainium2 NeuronCores.

    x1  = x @ W.T                      # dense projection [N, 128]
    out = relu(segment_sum(x1[src] * vals[:, None], dst, N))

Key identity: the projection commutes with the weighted segment-sum,
    out = relu(segment_sum(vals * x[src], dst, N) @ W.T)
so the kernel gathers RAW x rows (bf16) and applies W once per 128-node
output block after accumulation — no dense pre-pass, no x1 table round-trip.

Strategy (edge/data parallel SpMM, dst-bucketed so no collective needed):
  * dst space is cut into 128-node blocks. Blocks are assigned to
    (core, slot) pairs, balanced by edge count. Every core owns the
    complete output rows for its blocks => no all-reduce; the host
    just re-assembles the slices.
  * Each core gathers x rows for its edges in bulk with dma_gather
    (one SDMA descriptor per edge). dma_gather indices are int16, so the
    table is addressed in two halves (rows < 32768 and >= 32768) and each
    slot's edges are split into a lo and a hi section.
  * DVE work is batched: one is_equal builds the one-hot S1[e, i, n] =
    (dst_local[e, i] == n) for a whole slot (dl stored slot-major), and one
    multiply folds vals into each landed gather chunk (vl stored
    section-major), so DVE runs ~2 ops per slot/chunk instead of 2 per tile.
  * Per 128-edge tile: PSUM u[c, n] += Gs[e, c].T @ S1_i on the tensor
    engine (Gs = gathered raw x rows scaled by vals).
  * Per slot epilogue: u -> bf16 SBUF, z = W @ u on PE, ReLU on ACT,
    DMA out (z stored feature-major; host transposes on assemble).
  * SPMD: the instruction stream is identical on all cores; per-slot tile
    counts are shared across cores (max over the 8 blocks in the slot
    group), so only the DATA differs per core.
"""

import math

import numpy as np
import ml_dtypes

import concourse.bacc as bacc
import concourse.bass as bass
import concourse.mybir as mybir
import concourse.tile as tile
from concourse.bass_utils import run_bass_kernel_spmd

P = 128          # partitions / block size / edge-tile size
CH = 128         # in/out channels (problem-specific)
N_CORES = 8
H_SPLIT = 32768  # int16 index limit for dma_gather
GC = 48          # gather chunk size in tiles (6144 edges / call)
SINGLE_PACKET = False
GBUFS = 8        # gather pool buffers
GSBUFS = 3       # scaled-gather pool buffers

F32 = mybir.dt.float32
BF16 = mybir.dt.bfloat16
I32 = mybir.dt.int32
I16 = mybir.dt.int16


def _wrap_idx(idx):
    """int16 index layout for dma_gather: element i at partition i%16,
    column i//16; 16-partition block replicated to all 128 partitions."""
    L = len(idx) // 16
    w = idx.reshape(L, 16).T.astype(np.int16)  # [16, L]
    return np.ascontiguousarray(np.tile(w, (8, 1)))  # [128, L]


def _plan_edges(src, dst, vals, n_nodes, n_cores, h_split):
    """Bucket edges by 128-node dst block, assign blocks to (slot, core),
    split each slot's edges into lo (src < h_split) / hi sections, pad each
    (slot, core, section) to T*128 edges shared across cores."""
    nb = math.ceil(n_nodes / P)
    nb_pad = math.ceil(nb / n_cores) * n_cores
    slots = nb_pad // n_cores

    blk = (dst // P).astype(np.int64)
    counts = np.bincount(blk, minlength=nb_pad)
    # group blocks by (ceil(lo/128), ceil(hi/128)) profile: slot padding is
    # max-over-group of each section's tile count, so same-profile groups
    # waste nothing
    counts_lo = np.bincount(blk[src < h_split], minlength=nb_pad)
    counts_hi = counts - counts_lo
    cl = -(-counts_lo // P)
    chh = -(-counts_hi // P)
    order = np.lexsort((counts, -chh, -cl))

    assign = np.empty((slots, n_cores), dtype=np.int64)
    totals = np.zeros(n_cores, dtype=np.int64)
    for s in range(slots):
        group = order[s * n_cores:(s + 1) * n_cores]
        cs = np.argsort(totals, kind="stable")  # least-loaded cores first
        for i, b in enumerate(group):
            assign[s, cs[i]] = b
            totals[cs[i]] += counts[b]

    # per-edge-per-core grouping
    eorder = np.argsort(blk, kind="stable")
    starts = np.zeros(nb_pad + 1, dtype=np.int64)
    np.cumsum(counts, out=starts[1:])

    # edge lists per (slot, core, section)
    lists = [[None] * n_cores for _ in range(slots)]
    TL = np.zeros(slots, dtype=np.int64)
    TH = np.zeros(slots, dtype=np.int64)
    for s in range(slots):
        for c in range(n_cores):
            b = int(assign[s, c])
            e = eorder[starts[b]:starts[b + 1]]
            lo = e[src[e] < h_split]
            hi = e[src[e] >= h_split]
            # sort by src: descriptors then walk the x table monotonically,
            # turning random 256B HBM reads into row-local ones
            lo = lo[np.argsort(src[lo], kind="stable")]
            hi = hi[np.argsort(src[hi], kind="stable")]
            lists[s][c] = (lo, hi)
            TL[s] = max(TL[s], -(-len(lo) // P))
            TH[s] = max(TH[s], -(-len(hi) // P))
        if TL[s] + TH[s] == 0:
            TL[s] = 1  # keep the psum chain non-empty
    KL = int(TL.sum())
    KH = int(TH.sum())
    K = KL + KH
    offL = np.zeros(slots + 1, dtype=np.int64)
    np.cumsum(TL, out=offL[1:])
    offH = np.zeros(slots + 1, dtype=np.int64)
    np.cumsum(TH, out=offH[1:])

    # dl in SLOT-major column order (each slot's lo+hi tiles contiguous, so
    # one batched is_equal per slot); vl in SECTION-major order (matching
    # gather chunk order, so one batched multiply per gather chunk)
    srcL = np.zeros((n_cores, KL * P), dtype=np.int64)
    srcH = np.zeros((n_cores, KH * P), dtype=np.int64)
    dstl_a = np.zeros((n_cores, K * P), dtype=np.float32)
    val_a = np.zeros((n_cores, K * P), dtype=np.float32)
    for s in range(slots):
        mo = int(offL[s]) + int(offH[s])  # slot-major tile offset
        for c in range(n_cores):
            b = int(assign[s, c])
            lo, hi = lists[s][c]
            ll = int(offL[s]) * P
            srcL[c, ll:ll + len(lo)] = src[lo]
            dstl_a[c, mo * P:mo * P + len(lo)] = \
                (dst[lo] - b * P).astype(np.float32)
            val_a[c, ll:ll + len(lo)] = vals[lo]
            ho = int(offH[s]) * P
            srcH[c, ho:ho + len(hi)] = src[hi] - h_split
            mh = (mo + int(TL[s])) * P
            dstl_a[c, mh:mh + len(hi)] = (dst[hi] - b * P).astype(np.float32)
            hh = (KL + int(offH[s])) * P
            val_a[c, hh:hh + len(hi)] = vals[hi]

    idxL = np.stack([_wrap_idx(srcL[c]) for c in range(n_cores)]) \
        if KL else np.zeros((n_cores, P, 0), dtype=np.int16)
    idxH = np.stack([_wrap_idx(srcH[c]) for c in range(n_cores)]) \
        if KH else np.zeros((n_cores, P, 0), dtype=np.int16)

    # interleave dstl/vals: position j -> (tile j//P, partition j%P) => [P, K]
    dstl_i = np.ascontiguousarray(dstl_a.reshape(n_cores, K, P).transpose(0, 2, 1))
    val_i = np.ascontiguousarray(val_a.reshape(n_cores, K, P).transpose(0, 2, 1))
    meta = np.ascontiguousarray(
        np.concatenate([dstl_i, val_i], axis=2).astype(ml_dtypes.bfloat16))

    plan = {
        "assign": assign,
        "h": h_split,
        "slots": slots,
        "TL": TL.tolist(),
        "TH": TH.tolist(),
        "KL": KL,
        "KH": KH,
        "offL": offL.tolist(),
        "offH": offH.tolist(),
    }
    return plan, idxL, idxH, meta


def _build_nc(xrows, plan, n_cores, loop_n=1, mode="full"):
    """Build the SPMD Bass program (identical on every core).

    loop_n > 1 wraps the whole body in an on-device repeat loop; mode
    ("full" | "g" | "nog") ablates phases — both used only by the
    timing harness."""
    slots = plan["slots"]
    KL, KH = plan["KL"], plan["KH"]
    K = KL + KH

    nc = bacc.Bacc(
        "TRN2",
        target_bir_lowering=False,
        debug=False,
        enable_asserts=False,
        num_devices=n_cores,
        num_swdge_queues=4,
    )
    # raw x rows, node-major bf16 — this IS the gather table
    xr_d = nc.dram_tensor("xr", [xrows, CH], BF16, kind="ExternalInput").ap()
    wt_d = nc.dram_tensor("wt", [CH, CH], BF16, kind="ExternalInput").ap()
    # dstl and vals packed side by side so one DMA (one semaphore) loads both
    # (bf16: dst_local ints <= 127 are exact; vals round same as in the S mul)
    mt_d = nc.dram_tensor("meta", [P, 2 * K], BF16, kind="ExternalInput").ap()
    il_d = (
        nc.dram_tensor("idxlo", [P, KL * 8], I16, kind="ExternalInput").ap()
        if KL else None
    )
    ih_d = (
        nc.dram_tensor("idxhi", [P, KH * 8], I16, kind="ExternalInput").ap()
        if KH else None
    )
    # z stored feature-major per slot: [slots, CH, P] flattened
    out_d = nc.dram_tensor("out", [slots * CH, P], F32, kind="ExternalOutput").ap()

    with tile.TileContext(nc) as tc:
        if loop_n > 1:
            with tc.For_i(0, loop_n, 1):
                _emit_body(nc, tc, plan, xrows, xr_d, wt_d, mt_d, il_d, ih_d,
                           out_d, mode)
        else:
            _emit_body(nc, tc, plan, xrows, xr_d, wt_d, mt_d, il_d, ih_d,
                       out_d, mode)
    nc.compile()
    return nc


def _emit_body(nc, tc, plan, xrows, xr_d, wt_d, mt_d, il_d, ih_d, out_d,
               mode="full"):
    slots = plan["slots"]
    TL, TH = plan["TL"], plan["TH"]
    KL, KH = plan["KL"], plan["KH"]
    offL, offH = plan["offL"], plan["offH"]
    K = KL + KH
    with (
        tc.tile_pool(name="const", bufs=1) as constp,
        tc.tile_pool(name="gat", bufs=GBUFS) as gp,
        tc.tile_pool(name="gsc", bufs=GSBUFS) as gsp,
        tc.tile_pool(name="sel", bufs=4) as selp,
        tc.tile_pool(name="usb", bufs=3) as usbp,
        tc.tile_pool(name="res", bufs=3) as resp,
        tc.tile_pool(name="psu", bufs=4, space="PSUM") as psu,
        tc.tile_pool(name="psz", bufs=2, space="PSUM") as psz,
    ):
        # idx chunk tiles first: the first gather depends only on chunk 0's
        # small (~100KB) load, so it issues almost immediately
        nchunk = [-(-KL // GC) if KL else 0, -(-KH // GC) if KH else 0]
        idx_sb = {}
        for sec in (0, 1):
            ksec = KL if sec == 0 else KH
            srcd = il_d if sec == 0 else ih_d
            for cid in range(nchunk[sec]):
                nt = min(GC, ksec - cid * GC)
                t_ = constp.tile([P, nt * 8], I16, tag=f"idx{sec}_{cid}")
                nc.sync.dma_start(
                    out=t_[:], in_=srcd[:, cid * GC * 8:(cid * GC + nt) * 8])
                idx_sb[(sec, cid)] = (t_, nt)
        mt_sb = constp.tile([P, 2 * K], BF16)
        nc.sync.dma_start(out=mt_sb[:], in_=mt_d[:])
        dl_sb = mt_sb[:, :K]
        vl_sb = mt_sb[:, K:]
        wt_sb = constp.tile([CH, CH], BF16)
        nc.sync.dma_start(out=wt_sb[:], in_=wt_d[:])
        iota_i = constp.tile([P, P], I32)
        nc.gpsimd.iota(iota_i[:], pattern=[[1, P]], base=0, channel_multiplier=0)
        iota_f = constp.tile([P, P], BF16)
        nc.vector.tensor_copy(iota_f[:], iota_i[:])

        # chunked dma_gather per section; chunk tiles issued lazily.
        # After each chunk lands, one batched DVE multiply folds vals in:
        # Gs[p, t, c] = g[p, t, c] * vl[p, t]
        chunks = {}  # (sec, chunk_id) -> (scaled tile, tiles_in_chunk)
        qrr = [0]  # round-robin SWDGE queue so desc-gen uses all 4 Q7 pairs

        def chunk_of(sec, t):
            cid = t // GC
            key = (sec, cid)
            if key not in chunks:
                isb, nt = idx_sb[key]
                g = gp.tile([P, nt * CH], BF16, tag="gat")
                h = min(plan["h"], xrows)
                table = xr_d[:h, :] if sec == 0 else xr_d[h:, :]
                nc.gpsimd.dma_gather(
                    out_ap=g[:].rearrange("p (t c) -> p t c", c=CH),
                    in_ap=table,
                    idxs_ap=isb[:],
                    num_idxs=nt * P,
                    num_idxs_reg=nt * P,
                    elem_size=CH,
                    single_packet=SINGLE_PACKET,
                    queue_num=qrr[0],
                )
                qrr[0] = (qrr[0] + 1) % 4
                if mode == "g":
                    # keep the gather alive with a tiny consumer
                    dummy = selp.tile([P, 1], F32, tag="dmy")
                    nc.vector.tensor_copy(dummy[:], g[:, :1])
                    chunks[key] = (g, nt)
                    return chunks[key]
                vcol = cid * GC if sec == 0 else KL + cid * GC
                gs = gsp.tile([P, nt * CH], BF16, tag="gsc")
                nc.vector.tensor_tensor(
                    out=gs[:].rearrange("p (t c) -> p t c", c=CH),
                    in0=g[:].rearrange("p (t c) -> p t c", c=CH),
                    in1=vl_sb[:, vcol:vcol + nt].unsqueeze(-1)
                        .to_broadcast([P, nt, CH]),
                    op=mybir.AluOpType.mult,
                )
                chunks[key] = (gs, nt)
            return chunks[key]

        mo = 0  # slot-major tile offset into dl
        for s in range(slots):
            # unified tile ids: lo tiles then hi tiles of this slot
            tiles = [(0, offL[s] + t) for t in range(TL[s])]
            tiles += [(1, offH[s] + t) for t in range(TH[s])]
            T_s = len(tiles)
            if mode == "g":
                for sec, t in tiles:
                    chunk_of(sec, t)
                mo += T_s
                continue
            # batched one-hot: S1[p, i, n] = (dst_local[p, i] == n)
            S1 = selp.tile([P, T_s * P], BF16, tag="sel")
            nc.vector.tensor_tensor(
                out=S1[:].rearrange("p (i n) -> p i n", n=P),
                in0=dl_sb[:, mo:mo + T_s].unsqueeze(-1)
                    .to_broadcast([P, T_s, P]),
                in1=iota_f[:].unsqueeze(1).to_broadcast([P, T_s, P]),
                op=mybir.AluOpType.is_equal,
            )
            ps = psu.tile([CH, P], F32)
            for i, (sec, t) in enumerate(tiles):
                if mode == "nog":
                    g = wt_sb
                else:
                    g, _ = chunk_of(sec, t)
                # u[c, n] += Gs[e, c].T @ S1[e, n]
                nc.tensor.matmul(
                    out=ps[:],
                    lhsT=(wt_sb[:] if mode == "nog"
                          else g[:, (t % GC) * CH:(t % GC + 1) * CH]),
                    rhs=S1[:, i * P:(i + 1) * P],
                    start=(i == 0),
                    stop=(i == len(tiles) - 1),
                )
            mo += T_s
            # epilogue: z = W @ u, relu, out (feature-major)
            u_sb = usbp.tile([CH, P], BF16)
            nc.scalar.activation(
                out=u_sb[:], in_=ps[:],
                func=mybir.ActivationFunctionType.Copy)
            zs = psz.tile([CH, P], F32)
            nc.tensor.matmul(
                out=zs[:], lhsT=wt_sb[:], rhs=u_sb[:], start=True, stop=True)
            res = resp.tile([CH, P], F32)
            nc.scalar.activation(
                out=res[:], in_=zs[:], func=mybir.ActivationFunctionType.Relu
            )
            nc.sync.dma_start(out=out_d[s * CH:(s + 1) * CH, :], in_=res[:])


_NC_CACHE = {}


def prepare(x, W, src, dst, vals, n_cores=N_CORES, h_split=H_SPLIT):
    """Host-side planning + input maps."""
    x = np.asarray(x, dtype=np.float32)
    W = np.asarray(W, dtype=np.float32)
    src = np.asarray(src).astype(np.int64)
    dst = np.asarray(dst).astype(np.int64)
    vals = np.asarray(vals, dtype=np.float32)

    n = x.shape[0]
    plan, idxL, idxH, meta = _plan_edges(src, dst, vals, n, n_cores, h_split)

    xrows = n
    xr = np.ascontiguousarray(x.astype(ml_dtypes.bfloat16))  # [n, CH]
    wt = np.ascontiguousarray(W.T).astype(ml_dtypes.bfloat16)  # [c, o]

    key = (xrows, n_cores, plan["h"], plan["KL"], plan["KH"],
           tuple(plan["TL"]), tuple(plan["TH"]))
    nc = _NC_CACHE.get(key)
    if nc is None:
        nc = _build_nc(xrows, plan, n_cores)
        _NC_CACHE[key] = nc

    in_maps = []
    for c in range(n_cores):
        m = {"xr": xr, "wt": wt, "meta": meta[c]}
        if plan["KL"]:
            m["idxlo"] = idxL[c]
        if plan["KH"]:
            m["idxhi"] = idxH[c]
        in_maps.append(m)
    return nc, in_maps, plan, n


def assemble(results, plan, n, n_cores=N_CORES):
    """Scatter per-core slot outputs back to the full [n, CH] output."""
    assign, slots = plan["assign"], plan["slots"]
    out_full = np.zeros((slots * n_cores * P, CH), dtype=np.float32)
    for c in range(n_cores):
        o = results[c]["out"].reshape(slots, CH, P)
        for s in range(slots):
            b = int(assign[s, c])
            out_full[b * P:(b + 1) * P] = o[s].T
    return out_full[:n]


def kernel(x, W, src, dst, vals, **_run_kwargs):
    nc, in_maps, plan, n = prepare(x, W, src, dst, vals)
    res = run_bass_kernel_spmd(
        nc, in_maps, core_ids=list(range(N_CORES)), **_run_kwargs
    )
    out = assemble(res.results, plan, n)
    if _run_kwargs:
        return out, res
    return out

